# revision 6
# baseline (speedup 1.0000x reference)
"""Trainium2 Bass kernel for DSQG attention (J=12 causal-offset sparse attention).

Sharding: data-parallel over (B,H): 32 bh-slices -> 8 cores x 4 bh.
Each core processes its 4 bh as 2 stacked pairs in a transposed layout
[128 = 2bh x 64hd, N] so every sequence shift is a free-dim AP offset.

Per (pair, 1024-col chunk):
  scores: prod_i = (k_shift + se_i_col) * q   (one fused STT per offset)
          ones-block matmuls reduce the 64 hd partitions -> PSUM rows 32a+bh
          (a = i%4, bank g = i//4); exp via ACT (scale=1/8, bias=pb column);
          causal prefixes zeroed; denominator via 0/1-selector matmul + recip.
  values: B_i = e_i broadcast over hd via selector matmul (PSUM);
          acc += B_i * v_shift on DVE.
  rot:    theta on a 128-row stack (bh,i,t): t indexes the 8 per-offset product
          terms e*{cos-1,sin}(theta_p)*v_{ch}(n-d); a +/-1 selector matmul
          reduces them to the 4 rotated-channel corrections.
  final:  out = (acc + rot_corr) * (1/denom broadcast via selector matmul).
"""

import sys

for _p in ("/opt/trn_rl_repo", "/root/.axon_site/_ro/trn_rl_repo"):
    if _p not in sys.path:
        sys.path.insert(0, _p)

import numpy as np

OFFSETS = (1, 2, 4, 8, 16, 64, 96, 192, 384, 512, 768, 1024)
J = 12
B, H, N, HD = 2, 16, 4096, 64
PAD = 1024
NP_ = N + PAD
CH = 1024            # main chunk width
CHA = 512            # scores sub-chunk (PSUM bank budget)
NCHUNK = N // CH
SC = 1.0 / 8.0
NCORES = 8
ROT = OFFSETS[4:]    # 8 rotating offsets (abs i = 4..11)
T_P = (0, 0, 0, 0, 1, 1, 1, 1)      # phase pair per term slot t
T_CH = (0, 1, 0, 1, 2, 3, 2, 3)     # v channel per t
T_CS = (0, 0, 1, 1, 0, 0, 1, 1)     # 0 = cos branch, 1 = sin branch

_PROGRAM = None


def _build_program():
    import concourse.tile as tile
    from concourse import bacc, mybir

    f32 = mybir.dt.float32
    AluOp = mybir.AluOpType
    Act = mybir.ActivationFunctionType

    nc = bacc.Bacc()
    dp = nc.declare_dram_parameter

    ins = {}
    for s in range(2):
        ins[f"qT{s}"] = dp(f"qT{s}", [128, N], f32, isOutput=False)
        ins[f"kTp{s}"] = dp(f"kTp{s}", [128, NP_], f32, isOutput=False)
        ins[f"vTp{s}"] = dp(f"vTp{s}", [128, NP_], f32, isOutput=False)
        ins[f"y128_{s}"] = dp(f"y128_{s}", [128, N], f32, isOutput=False)
        ins[f"z128_{s}"] = dp(f"z128_{s}", [128, N], f32, isOutput=False)
        ins[f"vsh{s}"] = dp(f"vsh{s}", [128, N], f32, isOutput=False)
        ins[f"pbc{s}"] = dp(f"pbc{s}", [128, 3], f32, isOutput=False)
        ins[f"g128_{s}"] = dp(f"g128_{s}", [128, 1], f32, isOutput=False)
        ins[f"b128_{s}"] = dp(f"b128_{s}", [128, 1], f32, isOutput=False)
        ins[f"sec{s}"] = dp(f"sec{s}", [128, J], f32, isOutput=False)
    ins["ones2"] = dp("ones2", [128, 32], f32, isOutput=False)
    ins["esel"] = dp("esel", [128, 2], f32, isOutput=False)
    ins["bsel"] = dp("bsel", [128, 4 * 128], f32, isOutput=False)
    ins["rotsel"] = dp("rotsel", [128, 2 * 128], f32, isOutput=False)
    ins["rotred"] = dp("rotred", [128, 128], f32, isOutput=False)
    ins["rsel"] = dp("rsel", [2, 128], f32, isOutput=False)
    ins["cm1"] = dp("cm1", [128, 1], f32, isOutput=False)
    outs = [dp(f"outT{s}", [128, N], f32, isOutput=True) for s in range(2)]

    with tile.TileContext(nc) as tc:
        with (
            tc.tile_pool(name="consts", bufs=1) as cpool,
            tc.tile_pool(name="data", bufs=1) as dpool,
            tc.tile_pool(name="work", bufs=2) as wpool,
            tc.tile_pool(name="prods", bufs=3) as ppool,
            tc.tile_pool(name="psbig", bufs=1, space="PSUM") as psbig,
            tc.tile_pool(name="psmed", bufs=2, space="PSUM") as psmed,
        ):
            c_ones2 = cpool.tile([128, 32], f32, tag="c_ones2")
            nc.sync.dma_start(out=c_ones2, in_=ins["ones2"][:])
            c_esel = cpool.tile([128, 2], f32, tag="c_esel")
            nc.sync.dma_start(out=c_esel, in_=ins["esel"][:])
            c_bsel = cpool.tile([128, 4 * 128], f32, tag="c_bsel")
            nc.sync.dma_start(out=c_bsel, in_=ins["bsel"][:])
            c_rotsel = cpool.tile([128, 2 * 128], f32, tag="c_rotsel")
            nc.sync.dma_start(out=c_rotsel, in_=ins["rotsel"][:])
            c_rotred = cpool.tile([128, 128], f32, tag="c_rotred")
            nc.sync.dma_start(out=c_rotred, in_=ins["rotred"][:])
            c_rsel = cpool.tile([2, 128], f32, tag="c_rsel")
            nc.sync.dma_start(out=c_rsel, in_=ins["rsel"][:])
            c_cm1 = cpool.tile([128, 1], f32, tag="c_cm1")
            nc.sync.dma_start(out=c_cm1, in_=ins["cm1"][:])

            for s in range(2):
                qT = dpool.tile([128, N], f32, tag="qT")
                nc.sync.dma_start(out=qT, in_=ins[f"qT{s}"][:])
                kTp = dpool.tile([128, NP_], f32, tag="kTp")
                nc.sync.dma_start(out=kTp, in_=ins[f"kTp{s}"][:])
                vTp = dpool.tile([128, NP_], f32, tag="vTp")
                nc.sync.dma_start(out=vTp, in_=ins[f"vTp{s}"][:])
                vsh = dpool.tile([128, N], f32, tag="vsh")
                nc.sync.dma_start(out=vsh, in_=ins[f"vsh{s}"][:])
                c_pbc = cpool.tile([128, 3], f32, tag="c_pbc")
                nc.sync.dma_start(out=c_pbc, in_=ins[f"pbc{s}"][:])
                c_g128 = cpool.tile([128, 1], f32, tag="c_g128")
                nc.sync.dma_start(out=c_g128, in_=ins[f"g128_{s}"][:])
                c_b128 = cpool.tile([128, 1], f32, tag="c_b128")
                nc.sync.dma_start(out=c_b128, in_=ins[f"b128_{s}"][:])
                c_sec = cpool.tile([128, J], f32, tag="c_sec")
                nc.sync.dma_start(out=c_sec, in_=ins[f"sec{s}"][:])

                for c in range(NCHUNK):
                    n0 = c * CH
                    # ---------- [A] scores + exp + denom ----------
                    ec = wpool.tile([128, 3, CH], f32, tag="ec")
                    for half in range(2):
                        h0 = n0 + half * CHA
                        scps = psbig.tile([128, 3, CHA], f32, tag="scps")
                        for i, d in enumerate(OFFSETS):
                            a, g = i % 4, i // 4
                            prod = ppool.tile([128, CHA], f32, tag="prod")
                            nc.vector.scalar_tensor_tensor(
                                out=prod,
                                in0=kTp[:, PAD - d + h0: PAD - d + h0 + CHA],
                                scalar=c_sec[:, i: i + 1],
                                in1=qT[:, h0: h0 + CHA],
                                op0=AluOp.add,
                                op1=AluOp.mult,
                            )
                            nc.tensor.matmul(
                                out=scps[32 * a: 32 * a + 32, g, :],
                                lhsT=c_ones2,
                                rhs=prod,
                                start=True, stop=True,
                                tile_position=(0, 32 * a),
                            )
                        for g in range(3):
                            nc.scalar.activation(
                                out=ec[:, g, half * CHA: half * CHA + CHA],
                                in_=scps[:, g, :],
                                func=Act.Exp,
                                bias=c_pbc[:, g: g + 1],
                                scale=SC,
                            )
                    if c == 0:
                        for i, d in enumerate(OFFSETS):
                            a, g = i % 4, i // 4
                            nc.vector.memset(ec[32 * a: 32 * a + 2, g, 0:d], 0.0)
                    denps_t = psmed.tile([128, CH], f32, tag="med")
                    denps = denps_t[0:2, :]
                    for g in range(3):
                        for half in range(2):
                            nc.tensor.matmul(
                                out=denps[:, half * CHA: half * CHA + CHA],
                                lhsT=c_esel,
                                rhs=ec[:, g, half * CHA: half * CHA + CHA],
                                start=(g == 0), stop=(g == 2),
                            )
                    rc = wpool.tile([2, CH], f32, tag="rc")
                    nc.vector.tensor_scalar_add(rc, denps, 1e-30)
                    nc.vector.reciprocal(rc, rc)

                    # ---------- [D] plain value accumulation ----------
                    accc = wpool.tile([128, CH], f32, tag="accc")
                    for i, d in enumerate(OFFSETS):
                        a, g = i % 4, i // 4
                        bps = psmed.tile([128, CH], f32, tag="med")
                        for half in range(2):
                            nc.tensor.matmul(
                                out=bps[:, half * CHA: half * CHA + CHA],
                                lhsT=c_bsel[:, a * 128: a * 128 + 128],
                                rhs=ec[:, g, half * CHA: half * CHA + CHA],
                                start=True, stop=True,
                            )
                        vsl = vTp[:, PAD - d + n0: PAD - d + n0 + CH]
                        if i == 0:
                            nc.vector.tensor_mul(accc, bps, vsl)
                        else:
                            tmp = wpool.tile([128, CH], f32, tag="tmp")
                            nc.vector.tensor_mul(tmp, bps, vsl)
                            nc.vector.tensor_add(accc, accc, tmp)

                    # ---------- [R] rotation correction ----------
                    y128c = wpool.tile([128, CH], f32, tag="y128c")
                    nc.sync.dma_start(out=y128c, in_=ins[f"y128_{s}"][:, n0: n0 + CH])
                    z128c = wpool.tile([128, CH], f32, tag="z128c")
                    nc.sync.dma_start(out=z128c, in_=ins[f"z128_{s}"][:, n0: n0 + CH])
                    th = wpool.tile([128, CH], f32, tag="th")
                    nc.vector.tensor_mul(th, y128c, z128c)
                    nc.vector.tensor_scalar(
                        out=th, in0=th,
                        scalar1=c_g128[:, 0:1], scalar2=c_b128[:, 0:1],
                        op0=AluOp.mult, op1=AluOp.add,
                    )
                    nc.vector.add_range_wrap(th, th, 0.0, np.pi, 2.0 * np.pi)
                    nc.vector.add_range_wrap(th, th, 0.0, np.pi, 2.0 * np.pi)
                    trig = wpool.tile([128, CH], f32, tag="trig")
                    nc.scalar.activation(out=trig, in_=th, func=Act.Sin,
                                         bias=0.0, scale=1.0)
                    # cos rows -> cos - 1
                    nc.vector.tensor_scalar_add(trig, trig, c_cm1[:, 0:1])
                    erps = psmed.tile([128, CH], f32, tag="med")
                    for half in range(2):
                        for m in range(2):
                            nc.tensor.matmul(
                                out=erps[:, half * CHA: half * CHA + CHA],
                                lhsT=c_rotsel[:, m * 128: m * 128 + 128],
                                rhs=ec[:, 1 + m, half * CHA: half * CHA + CHA],
                                start=(m == 0), stop=(m == 1),
                            )
                    vful = wpool.tile([128, CH], f32, tag="vful")
                    nc.vector.tensor_mul(vful, erps, trig)
                    prot = wpool.tile([128, CH], f32, tag="prot")
                    nc.vector.tensor_mul(prot, vful, vsh[:, n0: n0 + CH])
                    rotps = psmed.tile([128, CH], f32, tag="med")
                    for half in range(2):
                        nc.tensor.matmul(
                            out=rotps[:, half * CHA: half * CHA + CHA],
                            lhsT=c_rotred,
                            rhs=prot[:, half * CHA: half * CHA + CHA],
                            start=True, stop=True,
                        )
                    nc.vector.tensor_add(accc, accc, rotps)

                    # ---------- [E] normalize + store ----------
                    rbps = psmed.tile([128, CH], f32, tag="med")
                    for half in range(2):
                        nc.tensor.matmul(
                            out=rbps[:, half * CHA: half * CHA + CHA],
                            lhsT=c_rsel,
                            rhs=rc[:, half * CHA: half * CHA + CHA],
                            start=True, stop=True,
                        )
                    outc = wpool.tile([128, CH], f32, tag="outc")
                    nc.vector.tensor_mul(outc, accc, rbps)
                    nc.sync.dma_start(out=outs[s][:, n0: n0 + CH], in_=outc)

    nc.compile()
    return nc


def get_program():
    global _PROGRAM
    if _PROGRAM is None:
        _PROGRAM = _build_program()
    return _PROGRAM


def _shift_np(x, d):
    """out[n] = x[n-d], zeros for n < d; shift along axis 0."""
    out = np.zeros_like(x)
    out[d:] = x[:-d] if d > 0 else x
    return out


def _shared_consts():
    c = {}
    ones2 = np.zeros((128, 32), np.float32)
    for bh in range(2):
        ones2[bh * 64:(bh + 1) * 64, bh] = 1.0
    c["ones2"] = ones2
    esel = np.zeros((128, 2), np.float32)
    for a in range(4):
        for bh in range(2):
            esel[32 * a + bh, bh] = 1.0
    c["esel"] = esel
    bsel = np.zeros((128, 4 * 128), np.float32)
    for a in range(4):
        for j in range(128):
            bsel[32 * a + j // 64, a * 128 + j] = 1.0
    c["bsel"] = bsel
    rotsel = np.zeros((128, 2 * 128), np.float32)
    for r in range(128):
        bh, i8 = r // 64, (r % 64) // 8
        abs_i = i8 + 4
        m = abs_i // 4 - 1          # 0 -> bank 1, 1 -> bank 2
        a = abs_i % 4
        rotsel[32 * a + bh, m * 128 + r] = 1.0
    c["rotsel"] = rotsel
    rotred = np.zeros((128, 128), np.float32)
    # corr[ch0] = sum_i P(t0) - P(t3); ch1 = P(t1) + P(t2)
    # corr[ch2] = P(t4) - P(t7);       ch3 = P(t5) + P(t6)
    sign_map = {0: ((0, 1.0), (3, -1.0)), 1: ((1, 1.0), (2, 1.0)),
                2: ((4, 1.0), (7, -1.0)), 3: ((5, 1.0), (6, 1.0))}
    for bh in range(2):
        for ch in range(4):
            col = bh * 64 + ch
            for i8 in range(8):
                for t, sgn in sign_map[ch]:
                    rotred[bh * 64 + i8 * 8 + t, col] = sgn
    c["rotred"] = rotred
    rsel = np.zeros((2, 128), np.float32)
    rsel[0, 0:64] = 1.0
    rsel[1, 64:128] = 1.0
    c["rsel"] = rsel
    cm1 = np.zeros((128, 1), np.float32)
    for r in range(128):
        if T_CS[r % 8] == 0:
            cm1[r, 0] = -1.0
    c["cm1"] = cm1
    return c


def _core_inputs(core, q, k, v, pb, se, phase_base, phase_gain, y_pre, z_pre,
                 shared):
    m = dict(shared)
    for s in range(2):
        bhs = [4 * core + 2 * s, 4 * core + 2 * s + 1]
        qT = np.zeros((128, N), np.float32)
        kTp = np.zeros((128, NP_), np.float32)
        vTp = np.zeros((128, NP_), np.float32)
        y128 = np.zeros((128, N), np.float32)
        z128 = np.zeros((128, N), np.float32)
        vsh = np.zeros((128, N), np.float32)
        g128 = np.zeros((128, 1), np.float32)
        b128 = np.zeros((128, 1), np.float32)
        pbc = np.zeros((128, 3), np.float32)
        sec = np.zeros((128, J), np.float32)
        for lbh, bh in enumerate(bhs):
            b, h = bh // H, bh % H
            r0 = lbh * 64
            qT[r0:r0 + 64, :] = q[b, h].T
            kTp[r0:r0 + 64, PAD:] = k[b, h].T
            vTp[r0:r0 + 64, PAD:] = v[b, h].T
            for i8, d in enumerate(ROT):
                for t in range(8):
                    r = r0 + i8 * 8 + t
                    p, ch = T_P[t], T_CH[t]
                    y128[r, :] = y_pre[b, h, :, p]
                    z128[r, :] = _shift_np(z_pre[b, h, :, p], d)
                    vsh[r, :] = _shift_np(v[b, h, :, ch], d)
                    g128[r, 0] = phase_gain[i8, h, p]
                    b128[r, 0] = phase_base[i8, h, p] + (
                        np.pi / 2.0 if T_CS[t] == 0 else 0.0)
            for i in range(J):
                a, g = i % 4, i // 4
                pbc[32 * a + lbh, g] = pb[i, h]
            sec[r0:r0 + 64, :] = se.T  # sec[r0+hd, i] = se[i, hd]
        m[f"qT{s}"] = qT
        m[f"kTp{s}"] = kTp
        m[f"vTp{s}"] = vTp
        m[f"y128_{s}"] = y128
        m[f"z128_{s}"] = z128
        m[f"vsh{s}"] = vsh
        m[f"g128_{s}"] = g128
        m[f"b128_{s}"] = b128
        m[f"pbc{s}"] = pbc
        m[f"sec{s}"] = sec
    return m


def make_in_maps(q, k, v, pb, se, phase_base, phase_gain, y_pre, z_pre):
    shared = _shared_consts()
    return [
        _core_inputs(c, q, k, v, pb, se, phase_base, phase_gain, y_pre, z_pre,
                     shared)
        for c in range(NCORES)
    ]


def assemble_output(results):
    out = np.zeros((B, H, N, HD), np.float32)
    for core in range(NCORES):
        for s in range(2):
            outT = results[core][f"outT{s}"]
            for lbh in range(2):
                bh = 4 * core + 2 * s + lbh
                b, h = bh // H, bh % H
                out[b, h] = outT[lbh * 64:(lbh + 1) * 64, :].T
    return out


def kernel(**inputs):
    from concourse.bass_utils import run_bass_kernel_spmd

    nc = get_program()
    in_maps = make_in_maps(**inputs)
    res = run_bass_kernel_spmd(nc, in_maps, core_ids=list(range(NCORES)))
    return assemble_output(res.results)


if __name__ == "__main__":
    get_program()
    print("program built + compiled OK")
